# revision 1
# baseline (speedup 1.0000x reference)
"""DeepseekV2 MLA attention on 8 Trainium2 NeuronCores (Bass/Tile), v4.

Token-sharded front end: each core computes q_a/latent + RMS-norm + RoPE and
the q_b/kv_b projections for ALL heads on its 256-token shard, then a single
AllToAll redistributes to head-sharded layout (2 heads/core, all tokens) for
attention and the row-parallel output projection. Host sums partials.
"""

import numpy as np

import concourse.bass as bass
import concourse.bacc as bacc
import concourse.mybir as mybir
import concourse.tile as tile
from concourse import bass_utils

T = 2048
HID = 2048
H = 16
DN = 128
DR = 64
DV = 128
DQK = DN + DR
QLR = 1536
KVLR = 512
THETA = 10000.0
EPS = 1e-6
SCALE = DQK ** -0.5

NCORES = 8
HPC = H // NCORES
LATR = KVLR + DR

F32 = mybir.dt.float32
F32R = mybir.dt.float32r

KT = HID // 128
QMT = QLR // 128
KVMT = KVLR // 128
NB = T // 512
TBT = T // 128
TSH = T // NCORES            # 256 tokens per shard

# per-dest chunk layouts for the two AllToAlls (rows):
# kv: kn h0 (128) | kn h1 (128) | v h0 (128) | v h1 (128) | kpe (64) = 576
# q:  qn h0 (128) | qn h1 (128) | qpe h0 (64) | qpe h1 (64)        = 384
KCH = 576
QCH = 384
OFF_KN = 0
OFF_V = 256
OFF_KPE = 512
OFF_QN = 0
OFF_QPE = 256


def build_bass():
    nc = bacc.Bacc(
        "TRN2",
        target_bir_lowering=False,
        debug=False,
        enable_asserts=False,
        num_devices=NCORES,
    )

    hs_sh = nc.dram_tensor("hs_sh", [HID, TSH], F32R, kind="ExternalInput").ap()
    wqa = nc.dram_tensor("wqa", [QMT * 128, KT * 128], F32R, kind="ExternalInput").ap()
    wkva = nc.dram_tensor("wkva", [KVMT * 128, KT * 128], F32R, kind="ExternalInput").ap()
    wkpe = nc.dram_tensor("wkpe", [128, KT * DR], F32R, kind="ExternalInput").ap()
    wqb = nc.dram_tensor("wqb", [NCORES * 128, QMT * HPC * DQK], F32R, kind="ExternalInput").ap()
    wkvbk = nc.dram_tensor("wkvbk", [NCORES * 128, KVMT * HPC * DN], F32R, kind="ExternalInput").ap()
    wkvbv = nc.dram_tensor("wkvbv", [NCORES * 128, KVMT * HPC * DV], F32R, kind="ExternalInput").ap()
    wo = nc.dram_tensor("wo", [HPC * DV, HID], F32R, kind="ExternalInput").ap()
    cosf_sh = nc.dram_tensor("cosf_sh", [DR, TSH], F32R, kind="ExternalInput").ap()
    sinf_sh = nc.dram_tensor("sinf_sh", [DR, TSH], F32R, kind="ExternalInput").ap()
    perm64 = nc.dram_tensor("perm64", [DR, DR], F32R, kind="ExternalInput").ap()
    ident = nc.dram_tensor("ident", [128, 128], F32R, kind="ExternalInput").ap()
    maskd = nc.dram_tensor("maskd", [128, 4 * 512], F32R, kind="ExternalInput").ap()
    ones = nc.dram_tensor("ones", [128, 128], F32R, kind="ExternalInput").ap()
    out = nc.dram_tensor("out", [T, HID], F32, kind="ExternalOutput").ap()

    with tile.TileContext(nc) as tc:
        _kernel_body(nc, tc, hs_sh, wqa, wkva, wkpe, wqb, wkvbk, wkvbv, wo,
                     cosf_sh, sinf_sh, perm64, ident, maskd, ones, out)

    nc.compile()
    return nc


def _kernel_body(nc, tc, hs_sh, wqa, wkva, wkpe, wqb, wkvbk, wkvbv, wo,
                 cosf_sh, sinf_sh, perm64, ident, maskd, ones, out):
    from contextlib import ExitStack

    ctx = ExitStack()
    with ctx:
        dram = ctx.enter_context(tc.tile_pool(name="dram", bufs=1, space="DRAM"))
        contrib_kv = dram.tile([NCORES * KCH, TSH], F32R)
        contrib_q = dram.tile([NCORES * QCH, TSH], F32R)
        a2a_kv = dram.tile([NCORES * KCH, TSH], F32R)
        a2a_q = dram.tile([NCORES * QCH, TSH], F32R)

        persist = ctx.enter_context(tc.tile_pool(name="persist", bufs=1))
        ones128 = persist.tile([128, 128], F32R, tag="ones128")
        nc.sync.dma_start(out=ones128, in_=ones)
        ones_col = ones128[:, 0:1]
        ones_row = ones128[0:1, :]
        perm_t = persist.tile([DR, DR], F32R, tag="perm0")
        nc.sync.dma_start(out=perm_t, in_=perm64)
        ident_t = persist.tile([128, 128], F32R, tag="ident")
        nc.sync.dma_start(out=ident_t, in_=ident)
        cosf_t = persist.tile([DR, TSH], F32R, tag="cosfsh")
        nc.sync.dma_start(out=cosf_t, in_=cosf_sh)
        sinf_t = persist.tile([DR, TSH], F32R, tag="sinfsh")
        nc.sync.dma_start(out=sinf_t, in_=sinf_sh)
        pmid = ctx.enter_context(tc.tile_pool(name="pmid", bufs=1))

        # ---- Phase A: shard q_a / latent, norm, local rope of k_pe ----------
        with tc.tile_pool(name="pa", bufs=1) as pa, \
             tc.tile_pool(name="psa", bufs=1, space="PSUM") as psa:
            hst = []
            for k in range(KT):
                h = pa.tile([128, TSH], F32R, tag=f"hs{k}")
                nc.sync.dma_start(out=h, in_=hs_sh[k * 128:(k + 1) * 128, :])
                hst.append(h)

            def a_mtile(w_src, mrows, z_tile, z_start, z_stop, stg_tag):
                wstrip = pa.tile([128, KT, mrows], F32R, tag="wstrip", bufs=2)
                nc.scalar.dma_start(
                    out=wstrip,
                    in_=w_src.rearrange("p (kc m) -> p kc m", kc=KT),
                )
                pq = psa.tile([mrows, TSH], F32, tag="pq", bufs=3)
                for k in range(KT):
                    nc.tensor.matmul(
                        pq, lhsT=wstrip[:, k, :], rhs=hst[k],
                        start=(k == 0), stop=(k == KT - 1))
                stage = pa.tile([mrows, TSH], F32R, tag=stg_tag, name=stg_tag)
                nc.vector.tensor_copy(stage, pq)
                if z_tile is not None:
                    sq = pa.tile([mrows, TSH], F32R, tag="sq", bufs=2)
                    nc.scalar.square(sq, stage)
                    nc.tensor.matmul(z_tile, lhsT=ones_col[0:mrows, :], rhs=sq,
                                     start=z_start, stop=z_stop)
                return stage

            def rsqrt_bc(z_psum, n, tag):
                tmp = pa.tile([1, TSH], F32, tag="rsq_tmp", bufs=2)
                nc.scalar.activation(tmp, z_psum,
                                     mybir.ActivationFunctionType.Copy,
                                     bias=EPS, scale=1.0 / n)
                nc.vector.reciprocal(tmp, tmp)
                srow = pa.tile([1, TSH], F32R, tag=tag + "r", name=tag + "r")
                nc.scalar.activation(srow, tmp,
                                     mybir.ActivationFunctionType.Sqrt)
                b_ps = psa.tile([128, TSH], F32, tag="bc", bufs=1)
                nc.tensor.matmul(b_ps, lhsT=ones_row, rhs=srow,
                                 start=True, stop=True)
                bc = pmid.tile([128, TSH], F32R, tag=tag, name=tag)
                nc.scalar.copy(bc, b_ps)
                return bc

            def rope_local(dst, raw, pool_ps):
                sw_ps = pool_ps.tile([DR, TSH], F32, tag="bc", bufs=1)
                nc.tensor.matmul(sw_ps, lhsT=perm_t, rhs=raw,
                                 start=True, stop=True)
                rt1 = pmid.tile([DR, TSH], F32R, tag="rt1", bufs=2)
                nc.vector.tensor_tensor(rt1, raw, cosf_t,
                                        op=mybir.AluOpType.mult)
                rt2 = pmid.tile([DR, TSH], F32R, tag="rt2", bufs=2)
                nc.vector.tensor_tensor(rt2, sw_ps, sinf_t,
                                        op=mybir.AluOpType.mult)
                nc.vector.tensor_tensor(dst, rt1, rt2, op=mybir.AluOpType.add)

            # kv side first
            zkv = psa.tile([1, TSH], F32, tag="z")
            kv_stages = []
            for m in range(KVMT):
                kv_stages.append(a_mtile(wkva[m * 128:(m + 1) * 128, :], 128,
                                         zkv, m == 0, m == KVMT - 1, f"stkv{m}"))
            kpe_stage = a_mtile(wkpe, DR, None, False, False, "stkpe")
            skv_bc = rsqrt_bc(zkv, KVLR, "skvbc")
            kvan = []
            for m in range(KVMT):
                kk = pmid.tile([128, TSH], F32R, tag=f"kvan{m}", name=f"kvan{m}")
                nc.vector.tensor_tensor(kk, kv_stages[m], skv_bc,
                                        op=mybir.AluOpType.mult)
                kvan.append(kk)
            kpel = pmid.tile([DR, TSH], F32R, tag="kpel")
            rope_local(kpel, kpe_stage, psa)
            # replicate roped k_pe into every dest chunk
            for d in range(NCORES):
                nc.sync.dma_start(
                    out=contrib_kv[d * KCH + OFF_KPE:d * KCH + OFF_KPE + DR, :],
                    in_=kpel)

            # q side
            zq = psa.tile([1, TSH], F32, tag="z")
            q_stages = []
            for m in range(QMT):
                q_stages.append(a_mtile(wqa[m * 128:(m + 1) * 128, :], 128,
                                        zq, m == 0, m == QMT - 1, f"stq{m}"))
            sq_bc = rsqrt_bc(zq, QLR, "sqbc")
            qan = []
            for m in range(QMT):
                qq = pmid.tile([128, TSH], F32R, tag=f"qan{m}", name=f"qan{m}")
                nc.vector.tensor_tensor(qq, q_stages[m], sq_bc,
                                        op=mybir.AluOpType.mult)
                qan.append(qq)

        # ---- kv_b projections for all dests + early kv exchange ----
        with tc.tile_pool(name="pw1", bufs=1) as pw1, \
             tc.tile_pool(name="psw1", bufs=1, space="PSUM") as psw1:
            for d in range(NCORES):
                wk = pw1.tile([128, KVMT, HPC * DN], F32R, tag="wk", bufs=2)
                nc.sync.dma_start(
                    out=wk,
                    in_=wkvbk[d * 128:(d + 1) * 128, :].rearrange(
                            "p (kc m) -> p kc m", kc=KVMT))
                wv = pw1.tile([128, KVMT, HPC * DV], F32R, tag="wv", bufs=2)
                nc.sync.dma_start(
                    out=wv,
                    in_=wkvbv[d * 128:(d + 1) * 128, :].rearrange(
                            "p (kc m) -> p kc m", kc=KVMT))
                for h in range(HPC):
                    acck = psw1.tile([128, TSH], F32, tag="acck", bufs=2,
                                     name="acck")
                    accv = psw1.tile([128, TSH], F32, tag="accv", bufs=2,
                                     name="accv")
                    for k in range(KVMT):
                        nc.tensor.matmul(
                            acck, lhsT=wk[:, k, h * DN:(h + 1) * DN],
                            rhs=kvan[k],
                            start=(k == 0), stop=(k == KVMT - 1))
                        nc.tensor.matmul(
                            accv, lhsT=wv[:, k, h * DV:(h + 1) * DV],
                            rhs=kvan[k],
                            start=(k == 0), stop=(k == KVMT - 1))
                    knt = pw1.tile([128, TSH], F32R, tag="knt", bufs=3)
                    nc.vector.tensor_copy(knt, acck)
                    nc.scalar.dma_start(
                        out=contrib_kv[d * KCH + OFF_KN + h * DN:
                                       d * KCH + OFF_KN + (h + 1) * DN, :],
                        in_=knt)
                    vtt = pw1.tile([128, TSH], F32R, tag="vtt", bufs=3)
                    nc.vector.tensor_copy(vtt, accv)
                    nc.scalar.dma_start(
                        out=contrib_kv[d * KCH + OFF_V + h * DV:
                                       d * KCH + OFF_V + (h + 1) * DV, :],
                        in_=vtt)
        nc.gpsimd.collective_compute(
            "AllToAll", mybir.AluOpType.bypass,
            replica_groups=[list(range(NCORES))],
            ins=[contrib_kv], outs=[a2a_kv])

        # ---- q_b projections for all dests + q exchange ------------------
        with tc.tile_pool(name="pw", bufs=1) as pw, \
             tc.tile_pool(name="psw", bufs=1, space="PSUM") as psw:
            for d in range(NCORES):
                wq = pw.tile([128, QMT, HPC * DQK], F32R, tag="wq", bufs=2)
                nc.sync.dma_start(
                    out=wq,
                    in_=wqb[d * 128:(d + 1) * 128, :].rearrange(
                        "p (kc m) -> p kc m", kc=QMT))
                col_of = (0, DN, 2 * DN, 2 * DN + DR)
                rows_of = (DN, DN, DR, DR)
                accq = []
                for mt in range(4):
                    a = psw.tile([rows_of[mt], TSH], F32, tag="acc",
                                 bufs=4, name=f"accq{mt}")
                    accq.append(a)
                for k in range(QMT):
                    for mt in range(4):
                        nc.tensor.matmul(
                            accq[mt],
                            lhsT=wq[:, k, col_of[mt]:col_of[mt] + rows_of[mt]],
                            rhs=qan[k],
                            start=(k == 0), stop=(k == QMT - 1))
                for h in range(HPC):
                    qnt = pw.tile([128, TSH], F32R, tag="qnt", bufs=3)
                    nc.vector.tensor_copy(qnt, accq[h])
                    nc.scalar.dma_start(
                        out=contrib_q[d * QCH + OFF_QN + h * DN:
                                      d * QCH + OFF_QN + (h + 1) * DN, :],
                        in_=qnt)
                    qraw = pw.tile([DR, TSH], F32R, tag="qraw", bufs=2)
                    nc.vector.tensor_copy(qraw, accq[2 + h])
                    qper = pw.tile([DR, TSH], F32R, tag="qper", bufs=3)
                    rope_local(qper, qraw, psw)
                    nc.scalar.dma_start(
                        out=contrib_q[d * QCH + OFF_QPE + h * DR:
                                      d * QCH + OFF_QPE + (h + 1) * DR, :],
                        in_=qper)
            nc.gpsimd.collective_compute(
                "AllToAll", mybir.AluOpType.bypass,
                replica_groups=[list(range(NCORES))],
                ins=[contrib_q], outs=[a2a_q])

        # ---- Phase B: load head-sharded tiles, transpose v ------------------
        qn = [[None] * NB for _ in range(HPC)]
        qpe = [[None] * NB for _ in range(HPC)]
        kn = [[None] * NB for _ in range(HPC)]
        kpe = [None] * NB
        vt = [None] * TBT

        bcp = ctx.enter_context(tc.tile_pool(name="bcp", bufs=1))

        with tc.tile_pool(name="pb", bufs=1) as pb, \
             tc.tile_pool(name="psb", bufs=1, space="PSUM") as psb:
            for j in range(NB):
                srcs = (2 * j, 2 * j + 1)
                for h in range(HPC):
                    qn[h][j] = bcp.tile([128, 512], F32R, tag=f"qn{h}_{j}",
                                        name=f"qn{h}_{j}")
                    qpe[h][j] = bcp.tile([DR, 512], F32R, tag=f"qpe{h}_{j}",
                                         name=f"qpe{h}_{j}")
                    kn[h][j] = bcp.tile([128, 512], F32R, tag=f"kn{h}_{j}",
                                        name=f"kn{h}_{j}")
                    for half, s in enumerate(srcs):
                        hsl = slice(half * TSH, (half + 1) * TSH)
                        nc.sync.dma_start(
                            out=kn[h][j][:, hsl],
                            in_=a2a_kv[s * KCH + OFF_KN + h * DN:
                                       s * KCH + OFF_KN + (h + 1) * DN, :])
                        nc.sync.dma_start(
                            out=qn[h][j][:, hsl],
                            in_=a2a_q[s * QCH + OFF_QN + h * DN:
                                      s * QCH + OFF_QN + (h + 1) * DN, :])
                        nc.scalar.dma_start(
                            out=qpe[h][j][:, hsl],
                            in_=a2a_q[s * QCH + OFF_QPE + h * DR:
                                      s * QCH + OFF_QPE + (h + 1) * DR, :])
                kpe[j] = bcp.tile([DR, 512], F32R, tag=f"kpe_{j}",
                                  name=f"kpe_{j}")
                for half, s in enumerate(srcs):
                    nc.scalar.dma_start(
                        out=kpe[j][:, half * TSH:(half + 1) * TSH],
                        in_=a2a_kv[s * KCH + OFF_KPE:s * KCH + OFF_KPE + DR, :])
                # v: feature-major per source -> transpose to token-major
                for half, s in enumerate(srcs):
                    vfs = []
                    for h in range(HPC):
                        vf = pb.tile([DV, TSH], F32R, tag="vf", bufs=4,
                                     name=f"vf{h}")
                        nc.sync.dma_start(
                            out=vf, in_=a2a_kv[s * KCH + OFF_V + h * DV:
                                               s * KCH + OFF_V + (h + 1) * DV, :])
                        vfs.append(vf)
                    for tt in range(2):
                        tb = j * 4 + half * 2 + tt
                        vt[tb] = bcp.tile([128, HPC * DV], F32R,
                                          tag=f"v_{tb}", name=f"v_{tb}")
                        for h in range(HPC):
                            tr = psb.tile([128, 128], F32R, tag="tr", bufs=4)
                            nc.tensor.transpose(
                                tr,
                                vfs[h][:, tt * 128:(tt + 1) * 128],
                                ident_t)
                            nc.vector.tensor_copy(
                                vt[tb][:, h * DV:(h + 1) * DV], tr)

        # ---- Attention + output projection ---------------------------------
        with tc.tile_pool(name="pc", bufs=1) as pc, \
             tc.tile_pool(name="psc", bufs=1, space="PSUM") as psc:
            maskd_t = pc.tile([128, 4 * 512], F32R, tag="maskd")
            nc.sync.dma_start(out=maskd_t, in_=maskd)
            wo_t = []
            for h in range(HPC):
                w = pc.tile([128, HID], F32R, tag=f"wo{h}")
                nc.sync.dma_start(out=w, in_=wo[h * DV:(h + 1) * DV, :])
                wo_t.append(w)

            attn_n = [[None] * NB for _ in range(HPC)]
            for qj in range(NB):
                nki = 4 * qj + 4
                for h in range(HPC):
                    attn_ps = psc.tile([128, 512], F32, tag="attn", bufs=2)
                    z_ps = psc.tile([1, 512], F32, tag="zr", bufs=1)
                    for ki in range(nki):
                        jb, sub = ki // 4, ki % 4
                        ksl = slice(sub * 128, (sub + 1) * 128)
                        s_ps = psc.tile([128, 512], F32, tag="s", bufs=3)
                        nc.tensor.matmul(s_ps, lhsT=kn[h][jb][:, ksl],
                                         rhs=qn[h][qj],
                                         start=True, stop=False)
                        nc.tensor.matmul(s_ps, lhsT=kpe[jb][:, ksl],
                                         rhs=qpe[h][qj],
                                         start=False, stop=True)
                        e = pc.tile([128, 512], F32R, tag="e", bufs=4)
                        nc.scalar.activation(e, s_ps,
                                             mybir.ActivationFunctionType.Exp)
                        if ki >= 4 * qj:
                            sub_d = ki - 4 * qj
                            nc.vector.tensor_tensor(
                                e, e, maskd_t[:, sub_d * 512:(sub_d + 1) * 512],
                                op=mybir.AluOpType.mult)
                        nc.tensor.matmul(z_ps, lhsT=ones_col, rhs=e,
                                         start=(ki == 0), stop=(ki == nki - 1))
                        nc.tensor.matmul(attn_ps,
                                         lhsT=vt[ki][:, h * DV:(h + 1) * DV],
                                         rhs=e,
                                         start=(ki == 0), stop=(ki == nki - 1))
                    rz = pc.tile([1, 512], F32R, tag="rz", bufs=2)
                    with nc.allow_low_precision(reason="fp32r softmax denom"):
                        nc.vector.reciprocal(rz, z_ps)
                    bc_ps = psc.tile([128, 512], F32, tag="s", bufs=3)
                    nc.tensor.matmul(bc_ps, lhsT=ones_row, rhs=rz,
                                     start=True, stop=True)
                    bc_sb = pc.tile([128, 512], F32R, tag="bcs", bufs=2)
                    nc.scalar.copy(bc_sb, bc_ps)
                    attn_n[h][qj] = bcp.tile([128, 512], F32R,
                                             tag=f"attn{h}_{qj}",
                                             name=f"attn{h}_{qj}")
                    nc.vector.tensor_tensor(attn_n[h][qj], attn_ps, bc_sb,
                                            op=mybir.AluOpType.mult)

                for tt in range(4):
                    tb = qj * 4 + tt
                    tsl = slice(tt * 128, (tt + 1) * 128)
                    o_row = pc.tile([128, HID], F32, tag="orow", bufs=2)
                    for hb in range(NB):
                        o_ps = psc.tile([128, 512], F32, tag="o", bufs=2)
                        for h in range(HPC):
                            nc.tensor.matmul(
                                o_ps,
                                lhsT=attn_n[h][qj][:, tsl],
                                rhs=wo_t[h][:, hb * 512:(hb + 1) * 512],
                                start=(h == 0),
                                stop=(h == HPC - 1),
                            )
                        nc.vector.tensor_copy(
                            o_row[:, hb * 512:(hb + 1) * 512], o_ps)
                    nc.scalar.dma_start(
                        out=out[tb * 128:(tb + 1) * 128, :], in_=o_row)


_NC_CACHE = {}


def _get_nc():
    if "nc" not in _NC_CACHE:
        _NC_CACHE["nc"] = build_bass()
    return _NC_CACHE["nc"]


def make_in_maps(positions, hidden_states, w_q_a, q_a_ln_w, w_q_b, w_kv_a,
                 kv_a_ln_w, w_kv_b, w_o):
    positions = np.asarray(positions)
    hidden_states = np.asarray(hidden_states, dtype=np.float32)
    w_q_a = np.asarray(w_q_a, dtype=np.float32)
    q_a_ln_w = np.asarray(q_a_ln_w, dtype=np.float32)
    w_q_b = np.asarray(w_q_b, dtype=np.float32)
    w_kv_a = np.asarray(w_kv_a, dtype=np.float32)
    kv_a_ln_w = np.asarray(kv_a_ln_w, dtype=np.float32)
    w_kv_b = np.asarray(w_kv_b, dtype=np.float32)
    w_o = np.asarray(w_o, dtype=np.float32)

    hs_t = np.ascontiguousarray(hidden_states.T)

    order = np.concatenate([np.arange(0, DR, 2), np.arange(1, DR, 2)])

    wkva_p = w_kv_a.copy()
    wkva_p[:, KVLR:] = w_kv_a[:, KVLR:][:, order]
    wkva_p = np.ascontiguousarray(wkva_p)

    inv_freq = 1.0 / (THETA ** (np.arange(0, DR, 2, dtype=np.float64) / DR))
    ang = positions.astype(np.float64)[:, None] * inv_freq[None, :]
    cosT = np.cos(ang).T.astype(np.float32)
    sinT = np.sin(ang).T.astype(np.float32)
    cosf = np.ascontiguousarray(np.concatenate([cosT, cosT], axis=0))
    sinf = np.ascontiguousarray(np.concatenate([-sinT, sinT], axis=0))

    perm = np.zeros((DR, DR), dtype=np.float32)
    for i in range(DR):
        perm[i, (i + DR // 2) % DR] = 1.0

    maskd = np.zeros((128, 4 * 512), dtype=np.float32)
    p = np.arange(128)[:, None]
    f = np.arange(512)[None, :]
    for sub in range(4):
        maskd[:, sub * 512:(sub + 1) * 512] = (p + 128 * sub <= f)
    maskd = np.ascontiguousarray(maskd)

    # all-heads b-weights, columns grouped per destination core
    wqb_all = np.concatenate([
        np.concatenate([
            w_q_b[:, h0 * DQK:h0 * DQK + DN],
            w_q_b[:, h1 * DQK:h1 * DQK + DN],
            w_q_b[:, h0 * DQK + DN:(h0 + 1) * DQK][:, order],
            w_q_b[:, h1 * DQK + DN:(h1 + 1) * DQK][:, order],
        ], axis=1)
        for h0, h1 in ((2 * d, 2 * d + 1) for d in range(NCORES))
    ], axis=1) * q_a_ln_w[:, None] * SCALE
    wkvbk_all = np.concatenate([
        w_kv_b[:, h * (DN + DV):h * (DN + DV) + DN] for h in range(H)
    ], axis=1) * kv_a_ln_w[:, None]
    wkvbv_all = np.concatenate([
        w_kv_b[:, h * (DN + DV) + DN:(h + 1) * (DN + DV)] for h in range(H)
    ], axis=1) * kv_a_ln_w[:, None]

    def pack(w, mrows):
        # [K, M] -> strip-major [nstrips*128, (K/128)*mrows]: each strip row-
        # contiguous so the device DMA is 128 fat descriptors
        Kd, Md = w.shape
        n = Md // mrows
        return np.ascontiguousarray(
            w.reshape(Kd // 128, 128, n, mrows).transpose(2, 1, 0, 3)
            .reshape(n * 128, (Kd // 128) * mrows).astype(np.float32))

    wqa_pk = pack(w_q_a, 128)
    wkva_pk = pack(wkva_p[:, :KVLR], 128)
    wkpe_pk = pack(wkva_p[:, KVLR:], DR)
    wqb_pk = pack(wqb_all.astype(np.float32), HPC * DQK)
    wkvbk_pk = pack(wkvbk_all.astype(np.float32), HPC * DN)
    wkvbv_pk = pack(wkvbv_all.astype(np.float32), HPC * DV)

    in_maps = []
    for c in range(NCORES):
        h0, h1 = HPC * c, HPC * c + 1
        wo_c = np.concatenate([
            w_o[h0 * DV:(h0 + 1) * DV, :],
            w_o[h1 * DV:(h1 + 1) * DV, :],
        ], axis=0)
        tsl = slice(c * TSH, (c + 1) * TSH)
        in_maps.append({
            "hs_sh": np.ascontiguousarray(hs_t[:, tsl]),
            "wqa": wqa_pk,
            "wkva": wkva_pk,
            "wkpe": wkpe_pk,
            "wqb": wqb_pk,
            "wkvbk": wkvbk_pk,
            "wkvbv": wkvbv_pk,
            "wo": np.ascontiguousarray(wo_c.astype(np.float32)),
            "cosf_sh": np.ascontiguousarray(cosf[:, tsl]),
            "sinf_sh": np.ascontiguousarray(sinf[:, tsl]),
            "perm64": perm,
            "ident": np.eye(128, dtype=np.float32),
            "maskd": maskd,
            "ones": np.ones((128, 128), dtype=np.float32),
        })
    return in_maps


def kernel(positions, hidden_states, w_q_a, q_a_ln_w, w_q_b, w_kv_a,
           kv_a_ln_w, w_kv_b, w_o):
    nc = _get_nc()
    in_maps = make_in_maps(positions, hidden_states, w_q_a, q_a_ln_w, w_q_b,
                           w_kv_a, kv_a_ln_w, w_kv_b, w_o)
    res = bass_utils.run_bass_kernel_spmd(nc, in_maps, core_ids=list(range(NCORES)))
    acc = np.zeros((T, HID), dtype=np.float32)
    for c in range(NCORES):
        acc += res.results[c]["out"]
    return acc



# revision 6
# speedup vs baseline: 1.4986x; 1.4986x over previous
"""DeepseekV2 MLA attention on 8 Trainium2 NeuronCores (Bass/Tile), v5.

bf16 datapath (f32 PSUM accumulation + f32 softmax stats).  Token-sharded
front end computes q_a/kv_a + RMS-norm + RoPE on its 256-token shard; the
512+64-row kv latent is AllGather'ed (small payload, starts early) and each
core expands kv_b for only its 2 heads over all 2048 tokens, overlapping the
q AllToAll (per-dest q_b projections with packed m-tiles).  Attention and the
row-parallel output projection are head-sharded; host sums bf16 partials.
"""

import numpy as np

import concourse.bass as bass
import concourse.bacc as bacc
import concourse.mybir as mybir
import concourse.tile as tile
from concourse import bass_utils

T = 2048
HID = 2048
H = 16
DN = 128
DR = 64
DV = 128
DQK = DN + DR
QLR = 1536
KVLR = 512
THETA = 10000.0
EPS = 1e-6
SCALE = DQK ** -0.5

NCORES = 8
HPC = H // NCORES            # 2 heads per core
TSH = T // NCORES            # 256 tokens per shard

BF = mybir.dt.bfloat16
F32 = mybir.dt.float32

KT = HID // 128              # 16 contraction strips for q_a/kv_a
QMT = QLR // 128             # 12 contraction strips for q_b
KVMT = KVLR // 128           # 4 contraction strips for kv_b
NB = T // 512                # 4 query blocks
TBT = T // 128               # 16 token tiles

LCH = KVLR + DR              # 576 latent rows per source in the gather
QCH = HPC * DQK              # 384 q rows per dest chunk: qn h0|qn h1|qpe pair


def build_bass():
    nc = bacc.Bacc(
        "TRN2",
        target_bir_lowering=False,
        debug=False,
        enable_asserts=False,
        num_devices=NCORES,
    )

    hs_sh = nc.dram_tensor("hs_sh", [HID, TSH], BF, kind="ExternalInput").ap()
    wqa = nc.dram_tensor("wqa", [QMT * 128, KT * 128], BF, kind="ExternalInput").ap()
    wkva = nc.dram_tensor("wkva", [KVMT * 128, KT * 128], BF, kind="ExternalInput").ap()
    wkpe = nc.dram_tensor("wkpe", [128, KT * DR], BF, kind="ExternalInput").ap()
    wqb = nc.dram_tensor("wqb", [NCORES * 128, QMT * QCH], BF, kind="ExternalInput").ap()
    wkvb = nc.dram_tensor("wkvb", [128, KVMT * 4 * DN], BF, kind="ExternalInput").ap()
    wo = nc.dram_tensor("wo", [HPC * DV, HID], BF, kind="ExternalInput").ap()
    cosf_sh = nc.dram_tensor("cosf_sh", [128, TSH], BF, kind="ExternalInput").ap()
    sinf_sh = nc.dram_tensor("sinf_sh", [128, TSH], BF, kind="ExternalInput").ap()
    perm128 = nc.dram_tensor("perm128", [128, 128], BF, kind="ExternalInput").ap()
    maskd = nc.dram_tensor("maskd", [128, 4 * 512], BF, kind="ExternalInput").ap()
    ones = nc.dram_tensor("ones", [128, 128], BF, kind="ExternalInput").ap()
    out = nc.dram_tensor("out", [T, HID], BF, kind="ExternalOutput").ap()

    with tile.TileContext(nc) as tc:
        _kernel_body(nc, tc, hs_sh, wqa, wkva, wkpe, wqb, wkvb, wo,
                     cosf_sh, sinf_sh, perm128, maskd, ones, out)

    nc.compile()
    return nc


def _kernel_body(nc, tc, hs_sh, wqa, wkva, wkpe, wqb, wkvb, wo,
                 cosf_sh, sinf_sh, perm128, maskd, ones, out):
    from contextlib import ExitStack

    MUL = mybir.AluOpType.mult
    ADD = mybir.AluOpType.add

    ctx = ExitStack()
    with ctx:
        dram = ctx.enter_context(tc.tile_pool(name="dram", bufs=1, space="DRAM"))
        contrib_lat = dram.tile([LCH, TSH], BF)
        gath_lat = dram.tile([NCORES * LCH, TSH], BF)
        contrib_q = dram.tile([NCORES * QCH, TSH], BF)
        a2a_q = dram.tile([NCORES * QCH, TSH], BF)

        persist = ctx.enter_context(tc.tile_pool(name="persist", bufs=1))
        ones_t = persist.tile([128, 128], BF, tag="ones")
        nc.sync.dma_start(out=ones_t, in_=ones)
        ones_col = ones_t[:, 0:1]
        ones_row = ones_t[0:1, :]
        perm_t = persist.tile([128, 128], BF, tag="perm")
        nc.sync.dma_start(out=perm_t, in_=perm128)
        cosf_t = persist.tile([128, TSH], BF, tag="cosf")
        nc.sync.dma_start(out=cosf_t, in_=cosf_sh)
        sinf_t = persist.tile([128, TSH], BF, tag="sinf")
        nc.sync.dma_start(out=sinf_t, in_=sinf_sh)
        maskd_t = persist.tile([128, 4 * 512], BF, tag="maskd")
        nc.gpsimd.dma_start(out=maskd_t, in_=maskd)
        wo_t = []
        for h in range(HPC):
            w = persist.tile([128, HID], BF, tag=f"wo{h}")
            nc.gpsimd.dma_start(out=w, in_=wo[h * DV:(h + 1) * DV, :])
            wo_t.append(w)

        pmid = ctx.enter_context(tc.tile_pool(name="pmid", bufs=1))
        bcp = ctx.enter_context(tc.tile_pool(name="bcp", bufs=1))

        # ---- Phase A: shard q_a / latent, norms, local rope -----------------
        with tc.tile_pool(name="pa", bufs=1) as pa, \
             tc.tile_pool(name="psa", bufs=1, space="PSUM") as psa:
            hst = []
            for k in range(KT):
                h = pa.tile([128, TSH], BF, tag=f"hs{k}")
                nc.sync.dma_start(out=h, in_=hs_sh[k * 128:(k + 1) * 128, :])
                hst.append(h)

            def a_mtile(w_src, mrows, z_tile, z_start, z_stop, stg_tag):
                wstrip = pa.tile([128, KT, mrows], BF, tag="wstrip", bufs=3)
                nc.scalar.dma_start(
                    out=wstrip,
                    in_=w_src.rearrange("p (kc m) -> p kc m", kc=KT),
                )
                pq = psa.tile([mrows, TSH], F32, tag="pq", bufs=3)
                for k in range(KT):
                    nc.tensor.matmul(
                        pq, lhsT=wstrip[:, k, :], rhs=hst[k],
                        start=(k == 0), stop=(k == KT - 1))
                stage = pa.tile([mrows, TSH], BF, tag=stg_tag, name=stg_tag)
                nc.vector.tensor_copy(stage, pq)
                if z_tile is not None:
                    sq = pa.tile([mrows, TSH], BF, tag="sq", bufs=2)
                    nc.scalar.square(sq, stage)
                    nc.tensor.matmul(z_tile, lhsT=ones_col[0:mrows, :], rhs=sq,
                                     start=z_start, stop=z_stop)
                return stage

            def rsqrt_bc(z_psum, n, tag):
                tmp = pa.tile([1, TSH], F32, tag="rsq_tmp", bufs=2)
                nc.scalar.activation(tmp, z_psum,
                                     mybir.ActivationFunctionType.Copy,
                                     bias=EPS, scale=1.0 / n)
                nc.vector.reciprocal(tmp, tmp)
                srow = pa.tile([1, TSH], BF, tag=tag + "r", name=tag + "r")
                nc.scalar.activation(srow, tmp,
                                     mybir.ActivationFunctionType.Sqrt)
                b_ps = psa.tile([128, TSH], F32, tag="bc", bufs=1)
                nc.tensor.matmul(b_ps, lhsT=ones_row, rhs=srow,
                                 start=True, stop=True)
                bc = pmid.tile([128, TSH], BF, tag=tag, name=tag)
                nc.scalar.copy(bc, b_ps)
                return bc

            def rope_local(dst, raw, pool_ps, rows):
                sw_ps = pool_ps.tile([rows, TSH], F32, tag="swp", bufs=2)
                nc.tensor.matmul(sw_ps, lhsT=perm_t[0:rows, 0:rows], rhs=raw,
                                 start=True, stop=True)
                rt1 = pmid.tile([rows, TSH], BF, tag="rt1", bufs=2)
                nc.vector.tensor_tensor(rt1, raw, cosf_t[0:rows, :], op=MUL)
                rt2 = pmid.tile([rows, TSH], BF, tag="rt2", bufs=2)
                nc.vector.tensor_tensor(rt2, sw_ps, sinf_t[0:rows, :], op=MUL)
                nc.vector.tensor_tensor(dst, rt1, rt2, op=ADD)

            # kv side first: latent norm + roped kpe -> AllGather
            zkv = psa.tile([1, TSH], F32, tag="z")
            kv_stages = []
            for m in range(KVMT):
                kv_stages.append(a_mtile(wkva[m * 128:(m + 1) * 128, :], 128,
                                         zkv, m == 0, m == KVMT - 1, f"stkv{m}"))
            kpe_stage = a_mtile(wkpe, DR, None, False, False, "stkpe")
            skv_bc = rsqrt_bc(zkv, KVLR, "skvbc")
            for m in range(KVMT):
                kk = pmid.tile([128, TSH], BF, tag=f"kvan{m}", name=f"kvan{m}")
                nc.vector.tensor_tensor(kk, kv_stages[m], skv_bc, op=MUL)
                nc.sync.dma_start(
                    out=contrib_lat[m * 128:(m + 1) * 128, :], in_=kk)
            kpel = pmid.tile([DR, TSH], BF, tag="kpel")
            rope_local(kpel, kpe_stage, psa, DR)
            nc.sync.dma_start(out=contrib_lat[KVLR:LCH, :], in_=kpel)
            nc.gpsimd.collective_compute(
                "AllGather", mybir.AluOpType.bypass,
                replica_groups=[list(range(NCORES))],
                ins=[contrib_lat], outs=[gath_lat])

            # q side
            zq = psa.tile([1, TSH], F32, tag="z")
            q_stages = []
            for m in range(QMT):
                q_stages.append(a_mtile(wqa[m * 128:(m + 1) * 128, :], 128,
                                        zq, m == 0, m == QMT - 1, f"stq{m}"))
            sq_bc = rsqrt_bc(zq, QLR, "sqbc")
            qan = []
            for m in range(QMT):
                qq = pmid.tile([128, TSH], BF, tag=f"qan{m}", name=f"qan{m}")
                nc.vector.tensor_tensor(qq, q_stages[m], sq_bc, op=MUL)
                qan.append(qq)

        # ---- q_b projections for all dests + q exchange ---------------------
        with tc.tile_pool(name="pw", bufs=1) as pw, \
             tc.tile_pool(name="psw", bufs=1, space="PSUM") as psw:

            def rope_q(dst, raw):
                sw_ps = psw.tile([128, TSH], F32, tag="swp", bufs=2)
                nc.tensor.matmul(sw_ps, lhsT=perm_t, rhs=raw,
                                 start=True, stop=True)
                rt1 = pw.tile([128, TSH], BF, tag="rt1", bufs=2)
                nc.vector.tensor_tensor(rt1, raw, cosf_t, op=MUL)
                rt2 = pw.tile([128, TSH], BF, tag="rt2", bufs=2)
                nc.vector.tensor_tensor(rt2, sw_ps, sinf_t, op=MUL)
                nc.vector.tensor_tensor(dst, rt1, rt2, op=ADD)

            for d in range(NCORES):
                wq = pw.tile([128, QMT, QCH], BF, tag="wq", bufs=2)
                nc.sync.dma_start(
                    out=wq,
                    in_=wqb[d * 128:(d + 1) * 128, :].rearrange(
                        "p (kc m) -> p kc m", kc=QMT))
                acc = []
                for mt in range(3):
                    acc.append(psw.tile([128, TSH], F32, tag=f"acc{mt}",
                                        bufs=2, name=f"acc{mt}"))
                for k in range(QMT):
                    for mt in range(3):
                        nc.tensor.matmul(
                            acc[mt],
                            lhsT=wq[:, k, mt * 128:(mt + 1) * 128],
                            rhs=qan[k],
                            start=(k == 0), stop=(k == QMT - 1))
                for h in range(HPC):
                    qnt = pw.tile([128, TSH], BF, tag="qnt", bufs=3)
                    nc.vector.tensor_copy(qnt, acc[h])
                    nc.gpsimd.dma_start(
                        out=contrib_q[d * QCH + h * DN:
                                      d * QCH + (h + 1) * DN, :],
                        in_=qnt)
                qraw = pw.tile([128, TSH], BF, tag="qraw", bufs=2)
                nc.vector.tensor_copy(qraw, acc[2])
                qper = pw.tile([128, TSH], BF, tag="qper", bufs=3)
                rope_q(qper, qraw)
                nc.gpsimd.dma_start(
                    out=contrib_q[d * QCH + 2 * DN:d * QCH + 2 * DN + 128, :],
                    in_=qper)
            nc.gpsimd.collective_compute(
                "AllToAll", mybir.AluOpType.bypass,
                replica_groups=[list(range(NCORES))],
                ins=[contrib_q], outs=[a2a_q])

        # ---- kv_b expansion for local heads over all tokens (overlaps a2a) --
        lat = []
        kn = []
        vt = []
        with tc.tile_pool(name="pkb", bufs=1) as pkb, \
             tc.tile_pool(name="pskb", bufs=1, space="PSUM") as pskb:
            wkvb_t = pkb.tile([128, KVMT, 4 * DN], BF, tag="wkvb")
            nc.scalar.dma_start(
                out=wkvb_t,
                in_=wkvb.rearrange("p (kc m) -> p kc m", kc=KVMT))
            for k in range(KVMT):
                lt = bcp.tile([128, T], BF, tag=f"lat{k}", name=f"lat{k}")
                for s in range(NCORES):
                    nc.scalar.dma_start(
                        out=lt[:, s * TSH:(s + 1) * TSH],
                        in_=gath_lat[s * LCH + k * 128:
                                     s * LCH + (k + 1) * 128, :])
                lat.append(lt)
            kpe_all = bcp.tile([DR, T], BF, tag="kpeall", name="kpeall")
            for s in range(NCORES):
                nc.scalar.dma_start(
                    out=kpe_all[:, s * TSH:(s + 1) * TSH],
                    in_=gath_lat[s * LCH + KVLR:s * LCH + LCH, :])

            for h in range(HPC):
                knt = bcp.tile([128, T], BF, tag=f"kn{h}", name=f"kn{h}")
                for c in range(NB):
                    ps = pskb.tile([128, 512], F32, tag="knps", bufs=2)
                    for k in range(KVMT):
                        nc.tensor.matmul(
                            ps, lhsT=wkvb_t[:, k, h * DN:(h + 1) * DN],
                            rhs=lat[k][:, c * 512:(c + 1) * 512],
                            start=(k == 0), stop=(k == KVMT - 1))
                    if c % 2 == 0:
                        nc.vector.tensor_copy(knt[:, c * 512:(c + 1) * 512], ps)
                    else:
                        nc.scalar.copy(knt[:, c * 512:(c + 1) * 512], ps)
                kn.append(knt)
            for tb in range(TBT):
                v = bcp.tile([128, HPC * DV], BF, tag=f"v{tb}", name=f"v{tb}")
                ps = pskb.tile([128, HPC * DV], F32, tag="vps", bufs=3)
                for k in range(KVMT):
                    nc.tensor.matmul(
                        ps, lhsT=lat[k][:, tb * 128:(tb + 1) * 128],
                        rhs=wkvb_t[:, k, 2 * DN:4 * DN],
                        start=(k == 0), stop=(k == KVMT - 1))
                if tb % 2 == 0:
                    nc.vector.tensor_copy(v, ps)
                else:
                    nc.scalar.copy(v, ps)
                vt.append(v)

        # ---- Phase B: head-sharded attention + output projection ------------
        qn = [[None] * NB for _ in range(HPC)]
        qpe = [[None] * NB for _ in range(HPC)]
        for qj in range(NB):
            for h in range(HPC):
                qn[h][qj] = bcp.tile([128, 512], BF, tag=f"qn{h}_{qj}",
                                     name=f"qn{h}_{qj}")
                qpe[h][qj] = bcp.tile([DR, 512], BF, tag=f"qpe{h}_{qj}",
                                      name=f"qpe{h}_{qj}")
                for half, s in enumerate((2 * qj, 2 * qj + 1)):
                    hsl = slice(half * TSH, (half + 1) * TSH)
                    nc.sync.dma_start(
                        out=qn[h][qj][:, hsl],
                        in_=a2a_q[s * QCH + h * DN:s * QCH + (h + 1) * DN, :])
                    nc.sync.dma_start(
                        out=qpe[h][qj][:, hsl],
                        in_=a2a_q[s * QCH + 2 * DN + h * DR:
                                  s * QCH + 2 * DN + (h + 1) * DR, :])

        with tc.tile_pool(name="pc", bufs=1) as pc, \
             tc.tile_pool(name="psc", bufs=1, space="PSUM") as psc:
            attn_n = [None] * HPC
            for qj in range(NB):
                nki = 4 * qj + 4
                for h in range(HPC):
                    attn_ps = psc.tile([128, 512], F32, tag="attn", bufs=2)
                    z_ps = psc.tile([1, 512], F32, tag="zr", bufs=1)
                    for ki in range(nki):
                        ksl = slice(ki * 128, (ki + 1) * 128)
                        s_ps = psc.tile([128, 512], F32, tag="s", bufs=3)
                        nc.tensor.matmul(s_ps, lhsT=kn[h][:, ksl],
                                         rhs=qn[h][qj],
                                         start=True, stop=False)
                        nc.tensor.matmul(s_ps, lhsT=kpe_all[:, ksl],
                                         rhs=qpe[h][qj],
                                         start=False, stop=True)
                        e = pc.tile([128, 512], BF, tag="e", bufs=4)
                        nc.scalar.activation(e, s_ps,
                                             mybir.ActivationFunctionType.Exp)
                        if ki >= 4 * qj:
                            sub_d = ki - 4 * qj
                            nc.vector.tensor_tensor(
                                e, e, maskd_t[:, sub_d * 512:(sub_d + 1) * 512],
                                op=MUL)
                        nc.tensor.matmul(z_ps, lhsT=ones_col, rhs=e,
                                         start=(ki == 0), stop=(ki == nki - 1))
                        nc.tensor.matmul(attn_ps,
                                         lhsT=vt[ki][:, h * DV:(h + 1) * DV],
                                         rhs=e,
                                         start=(ki == 0), stop=(ki == nki - 1))
                    rz = pc.tile([1, 512], BF, tag="rz", bufs=2)
                    with nc.allow_low_precision(reason="bf16 softmax denom"):
                        nc.vector.reciprocal(rz, z_ps)
                    bc_ps = psc.tile([128, 512], F32, tag="s", bufs=3)
                    nc.tensor.matmul(bc_ps, lhsT=ones_row, rhs=rz,
                                     start=True, stop=True)
                    bc_sb = pc.tile([128, 512], BF, tag="bcs", bufs=2)
                    nc.scalar.copy(bc_sb, bc_ps)
                    attn_n[h] = pc.tile([128, 512], BF, tag=f"attnn{h}",
                                        bufs=2, name=f"attnn{h}_{qj}")
                    nc.vector.tensor_tensor(attn_n[h], attn_ps, bc_sb, op=MUL)

                for tt in range(4):
                    tb = qj * 4 + tt
                    tsl = slice(tt * 128, (tt + 1) * 128)
                    o_row = pc.tile([128, HID], BF, tag="orow", bufs=2)
                    for hb in range(NB):
                        o_ps = psc.tile([128, 512], F32, tag="o", bufs=2)
                        for h in range(HPC):
                            nc.tensor.matmul(
                                o_ps,
                                lhsT=attn_n[h][:, tsl],
                                rhs=wo_t[h][:, hb * 512:(hb + 1) * 512],
                                start=(h == 0),
                                stop=(h == HPC - 1),
                            )
                        if hb % 2 == 0:
                            nc.vector.tensor_copy(
                                o_row[:, hb * 512:(hb + 1) * 512], o_ps)
                        else:
                            nc.scalar.copy(
                                o_row[:, hb * 512:(hb + 1) * 512], o_ps)
                    if tt % 2 == 0:
                        nc.scalar.dma_start(
                            out=out[tb * 128:(tb + 1) * 128, :], in_=o_row)
                    else:
                        nc.sync.dma_start(
                            out=out[tb * 128:(tb + 1) * 128, :], in_=o_row)


_NC_CACHE = {}


def _get_nc():
    if "nc" not in _NC_CACHE:
        _NC_CACHE["nc"] = build_bass()
    return _NC_CACHE["nc"]


def make_in_maps(positions, hidden_states, w_q_a, q_a_ln_w, w_q_b, w_kv_a,
                 kv_a_ln_w, w_kv_b, w_o):
    BF_NP = mybir.dt.np(mybir.dt.bfloat16)

    positions = np.asarray(positions)
    hidden_states = np.asarray(hidden_states, dtype=np.float32)
    w_q_a = np.asarray(w_q_a, dtype=np.float32)
    q_a_ln_w = np.asarray(q_a_ln_w, dtype=np.float32)
    w_q_b = np.asarray(w_q_b, dtype=np.float32)
    w_kv_a = np.asarray(w_kv_a, dtype=np.float32)
    kv_a_ln_w = np.asarray(kv_a_ln_w, dtype=np.float32)
    w_kv_b = np.asarray(w_kv_b, dtype=np.float32)
    w_o = np.asarray(w_o, dtype=np.float32)

    hs_t = np.ascontiguousarray(hidden_states.T)

    # deinterleave rope features: evens then odds (dot-products invariant)
    order = np.concatenate([np.arange(0, DR, 2), np.arange(1, DR, 2)])

    wkva_p = w_kv_a.copy()
    wkva_p[:, KVLR:] = w_kv_a[:, KVLR:][:, order]

    inv_freq = 1.0 / (THETA ** (np.arange(0, DR, 2, dtype=np.float64) / DR))
    ang = positions.astype(np.float64)[:, None] * inv_freq[None, :]
    cosT = np.cos(ang).T
    sinT = np.sin(ang).T
    cosf = np.concatenate([cosT, cosT], axis=0)        # [64, T]
    sinf = np.concatenate([-sinT, sinT], axis=0)       # [64, T]
    cosf2 = np.concatenate([cosf, cosf], axis=0)       # [128, T] dual-head
    sinf2 = np.concatenate([sinf, sinf], axis=0)

    perm64 = np.zeros((DR, DR), dtype=np.float32)
    for i in range(DR):
        perm64[i, (i + DR // 2) % DR] = 1.0
    perm128 = np.zeros((128, 128), dtype=np.float32)
    perm128[:DR, :DR] = perm64
    perm128[DR:, DR:] = perm64

    maskd = np.zeros((128, 4 * 512), dtype=np.float32)
    p = np.arange(128)[:, None]
    f = np.arange(512)[None, :]
    for sub in range(4):
        maskd[:, sub * 512:(sub + 1) * 512] = (p + 128 * sub <= f)

    # all-heads q_b weights, columns grouped per destination core:
    # [qn h0 (128) | qn h1 (128) | qpe h0 perm (64) | qpe h1 perm (64)]
    wqb_all = np.concatenate([
        np.concatenate([
            w_q_b[:, h0 * DQK:h0 * DQK + DN],
            w_q_b[:, h1 * DQK:h1 * DQK + DN],
            w_q_b[:, h0 * DQK + DN:(h0 + 1) * DQK][:, order],
            w_q_b[:, h1 * DQK + DN:(h1 + 1) * DQK][:, order],
        ], axis=1)
        for h0, h1 in ((2 * d, 2 * d + 1) for d in range(NCORES))
    ], axis=1) * q_a_ln_w[:, None] * SCALE

    def pack(w, mrows):
        # [K, M] -> strip-major [nstrips*128, (K/128)*mrows]: each strip row-
        # contiguous so the device DMA is 128 fat descriptors
        Kd, Md = w.shape
        n = Md // mrows
        return np.ascontiguousarray(
            w.reshape(Kd // 128, 128, n, mrows).transpose(2, 1, 0, 3)
            .reshape(n * 128, (Kd // 128) * mrows)).astype(BF_NP)

    wqa_pk = pack(w_q_a, 128)
    wkva_pk = pack(wkva_p[:, :KVLR], 128)
    wkpe_pk = pack(wkva_p[:, KVLR:], DR)
    wqb_pk = pack(wqb_all, QCH)

    in_maps = []
    for c in range(NCORES):
        h0, h1 = HPC * c, HPC * c + 1
        # per-core kv_b: cols [kn h0 | kn h1 | v h0 | v h1], ln folded
        wkvb_c = np.concatenate([
            w_kv_b[:, h0 * (DN + DV):h0 * (DN + DV) + DN],
            w_kv_b[:, h1 * (DN + DV):h1 * (DN + DV) + DN],
            w_kv_b[:, h0 * (DN + DV) + DN:(h0 + 1) * (DN + DV)],
            w_kv_b[:, h1 * (DN + DV) + DN:(h1 + 1) * (DN + DV)],
        ], axis=1) * kv_a_ln_w[:, None]
        wkvb_pk = pack(wkvb_c, 4 * DN)
        wo_c = np.concatenate([
            w_o[h0 * DV:(h0 + 1) * DV, :],
            w_o[h1 * DV:(h1 + 1) * DV, :],
        ], axis=0)
        tsl = slice(c * TSH, (c + 1) * TSH)
        in_maps.append({
            "hs_sh": np.ascontiguousarray(hs_t[:, tsl]).astype(BF_NP),
            "wqa": wqa_pk,
            "wkva": wkva_pk,
            "wkpe": wkpe_pk,
            "wqb": wqb_pk,
            "wkvb": wkvb_pk,
            "wo": np.ascontiguousarray(wo_c).astype(BF_NP),
            "cosf_sh": np.ascontiguousarray(cosf2[:, tsl]).astype(BF_NP),
            "sinf_sh": np.ascontiguousarray(sinf2[:, tsl]).astype(BF_NP),
            "perm128": perm128.astype(BF_NP),
            "maskd": np.ascontiguousarray(maskd).astype(BF_NP),
            "ones": np.ones((128, 128), dtype=np.float32).astype(BF_NP),
        })
    return in_maps


def kernel(positions, hidden_states, w_q_a, q_a_ln_w, w_q_b, w_kv_a,
           kv_a_ln_w, w_kv_b, w_o):
    nc = _get_nc()
    in_maps = make_in_maps(positions, hidden_states, w_q_a, q_a_ln_w, w_q_b,
                           w_kv_a, kv_a_ln_w, w_kv_b, w_o)
    res = bass_utils.run_bass_kernel_spmd(nc, in_maps, core_ids=list(range(NCORES)))
    acc = np.zeros((T, HID), dtype=np.float32)
    for c in range(NCORES):
        acc += np.asarray(res.results[c]["out"], dtype=np.float32)
    return acc


# revision 8
# speedup vs baseline: 1.5740x; 1.0503x over previous
"""DeepseekV2 MLA attention on 8 Trainium2 NeuronCores (Bass/Tile), v6.

bf16 datapath (f32 PSUM accumulation + f32 softmax stats).  Token-sharded
front end computes q_a/kv_a + RMS-norm + RoPE on its 256-token shard; the
512+64-row kv latent is AllGather'ed (small payload, starts early) and each
core expands kv_b for only its 2 heads over all 2048 tokens, overlapping the
q AllToAll (per-dest q_b projections with packed m-tiles).  Attention and the
row-parallel output projection are head-sharded; host sums bf16 partials.

Collectives are issued at outer scope (a tile-pool close gates the next
pool's SBUF reuse on every instruction in the scope, so an in-scope
collective serializes the whole kernel).  All bulk HBM traffic moves in
batched DMAs (~1.3us fixed cost per DMA): contributions live in [128, free]
packed layouts so one descriptor-fat DMA moves each of them.
"""

import numpy as np

import concourse.bass as bass
import concourse.bacc as bacc
import concourse.mybir as mybir
import concourse.tile as tile
from concourse import bass_utils

T = 2048
HID = 2048
H = 16
DN = 128
DR = 64
DV = 128
DQK = DN + DR
QLR = 1536
KVLR = 512
THETA = 10000.0
EPS = 1e-6
SCALE = DQK ** -0.5

NCORES = 8
HPC = H // NCORES            # 2 heads per core
TSH = T // NCORES            # 256 tokens per shard

BF = mybir.dt.bfloat16
F32 = mybir.dt.float32

KT = HID // 128              # 16 contraction strips for q_a/kv_a
QMT = QLR // 128             # 12 contraction strips for q_b
KVMT = KVLR // 128           # 4 contraction strips for kv_b
NB = T // 512                # 4 query blocks
TBT = T // 128               # 16 token tiles

# packed contribution layouts, everything [128 partitions, free]
# latent: cols m*TSH..(m+1)*TSH = kv latent strip m (m<4); cols 4*TSH..4*TSH+128
#         = roped kpe packed as [64,256]->[128,128] (partitions 64:128 hold
#         the second 128 tokens)
LFC = KVMT * TSH + TSH // 2  # 1152 free cols per source
# q: per dest 3*TSH cols: qn h0 | qn h1 | qpe pair ([128,256]: partition
#    64*h+pa holds head h rope feature pa)
QFC = 3 * TSH                # 768 free cols per dest


def build_bass():
    nc = bacc.Bacc(
        "TRN2",
        target_bir_lowering=False,
        debug=False,
        enable_asserts=False,
        num_devices=NCORES,
    )

    hs_sh = nc.dram_tensor("hs_sh", [HID, TSH], BF, kind="ExternalInput").ap()
    wqa = nc.dram_tensor("wqa", [QMT * 128, KT * 128], BF, kind="ExternalInput").ap()
    wkva = nc.dram_tensor("wkva", [KVMT * 128, KT * 128], BF, kind="ExternalInput").ap()
    wkpe = nc.dram_tensor("wkpe", [128, KT * DR], BF, kind="ExternalInput").ap()
    wqb = nc.dram_tensor("wqb", [NCORES * 128, QMT * HPC * DQK], BF, kind="ExternalInput").ap()
    wkvb = nc.dram_tensor("wkvb", [128, KVMT * 4 * DN], BF, kind="ExternalInput").ap()
    wo = nc.dram_tensor("wo", [HPC * DV, HID], BF, kind="ExternalInput").ap()
    cosf_sh = nc.dram_tensor("cosf_sh", [128, TSH], BF, kind="ExternalInput").ap()
    sinf_sh = nc.dram_tensor("sinf_sh", [128, TSH], BF, kind="ExternalInput").ap()
    perm128 = nc.dram_tensor("perm128", [128, 128], BF, kind="ExternalInput").ap()
    maskd = nc.dram_tensor("maskd", [128, 4 * 512], BF, kind="ExternalInput").ap()
    ones = nc.dram_tensor("ones", [128, 128], BF, kind="ExternalInput").ap()
    out = nc.dram_tensor("out", [T, HID], BF, kind="ExternalOutput").ap()

    with tile.TileContext(nc) as tc:
        _kernel_body(nc, tc, hs_sh, wqa, wkva, wkpe, wqb, wkvb, wo,
                     cosf_sh, sinf_sh, perm128, maskd, ones, out)

    nc.compile()
    return nc


def _kernel_body(nc, tc, hs_sh, wqa, wkva, wkpe, wqb, wkvb, wo,
                 cosf_sh, sinf_sh, perm128, maskd, ones, out):
    from contextlib import ExitStack

    MUL = mybir.AluOpType.mult
    ADD = mybir.AluOpType.add

    ctx = ExitStack()
    with ctx:
        dram = ctx.enter_context(tc.tile_pool(name="dram", bufs=1, space="DRAM"))
        contrib_lat = dram.tile([128, LFC], BF)
        gath_lat = dram.tile([NCORES * 128, LFC], BF)
        contrib_q = dram.tile([NCORES * 128, QFC], BF)
        a2a_q = dram.tile([NCORES * 128, QFC], BF)

        persist = ctx.enter_context(tc.tile_pool(name="persist", bufs=1))
        ones_t = persist.tile([128, 128], BF, tag="ones")
        nc.sync.dma_start(out=ones_t, in_=ones)
        ones_col = ones_t[:, 0:1]
        ones_row = ones_t[0:1, :]
        perm_t = persist.tile([128, 128], BF, tag="perm")
        nc.sync.dma_start(out=perm_t, in_=perm128)
        cosf_t = persist.tile([128, TSH], BF, tag="cosf")
        nc.sync.dma_start(out=cosf_t, in_=cosf_sh)
        sinf_t = persist.tile([128, TSH], BF, tag="sinf")
        nc.sync.dma_start(out=sinf_t, in_=sinf_sh)
        maskd_t = persist.tile([128, 4 * 512], BF, tag="maskd")
        nc.gpsimd.dma_start(out=maskd_t, in_=maskd)
        wo_t = []
        for h in range(HPC):
            w = persist.tile([128, HID], BF, tag=f"wo{h}")
            nc.gpsimd.dma_start(out=w, in_=wo[h * DV:(h + 1) * DV, :])
            wo_t.append(w)

        pmid = ctx.enter_context(tc.tile_pool(name="pmid", bufs=1))
        bcp = ctx.enter_context(tc.tile_pool(name="bcp", bufs=1))

        # staging tiles for the packed contributions (persistent pools so the
        # collective's DMA sources outlive the producing scope)
        lat_stage = pmid.tile([128, LFC], BF, tag="latstage", name="lat_stage")
        q_stage = pmid.tile([128, NCORES * QFC], BF, tag="qstage",
                            name="q_stage")

        # ---- Phase A: shard q_a / latent, norms, local rope -----------------
        with tc.tile_pool(name="pa", bufs=1) as pa, \
             tc.tile_pool(name="psa", bufs=1, space="PSUM") as psa:
            hs_t = pa.tile([128, KT, TSH], BF, tag="hst")
            nc.sync.dma_start(
                out=hs_t, in_=hs_sh.rearrange("(kc p) t -> p kc t", kc=KT))
            wkva_t = pa.tile([128, KVMT, KT * 128], BF, tag="wkva")
            nc.scalar.dma_start(
                out=wkva_t,
                in_=wkva.rearrange("(n p) km -> p n km", n=KVMT))
            wkpe_t = pa.tile([128, KT, DR], BF, tag="wkpe")
            nc.scalar.dma_start(
                out=wkpe_t, in_=wkpe.rearrange("p (kc m) -> p kc m", kc=KT))
            wqa_t = []
            for g in range(3):
                wt = pa.tile([128, 4, KT * 128], BF, tag=f"wqa{g}")
                nc.scalar.dma_start(
                    out=wt,
                    in_=wqa[g * 4 * 128:(g + 1) * 4 * 128, :].rearrange(
                        "(n p) km -> p n km", n=4))
                wqa_t.append(wt)

            def a_mtile(lhs_of, mrows, z_tile, z_start, z_stop, stg_tag):
                pq = psa.tile([mrows, TSH], F32, tag="pq", bufs=3)
                for k in range(KT):
                    nc.tensor.matmul(
                        pq, lhsT=lhs_of(k), rhs=hs_t[:, k, :],
                        start=(k == 0), stop=(k == KT - 1))
                stage = pa.tile([mrows, TSH], BF, tag=stg_tag, name=stg_tag)
                nc.vector.tensor_copy(stage, pq)
                if z_tile is not None:
                    sq = pa.tile([mrows, TSH], BF, tag="sq", bufs=2)
                    nc.scalar.square(sq, stage)
                    nc.tensor.matmul(z_tile, lhsT=ones_col[0:mrows, :], rhs=sq,
                                     start=z_start, stop=z_stop)
                return stage

            def rsqrt_bc(z_psum, n, tag):
                tmp = pa.tile([1, TSH], F32, tag="rsq_tmp", bufs=2)
                nc.scalar.activation(tmp, z_psum,
                                     mybir.ActivationFunctionType.Copy,
                                     bias=EPS, scale=1.0 / n)
                nc.vector.reciprocal(tmp, tmp)
                srow = pa.tile([1, TSH], BF, tag=tag + "r", name=tag + "r")
                nc.scalar.activation(srow, tmp,
                                     mybir.ActivationFunctionType.Sqrt)
                b_ps = psa.tile([128, TSH], F32, tag="bc", bufs=1)
                nc.tensor.matmul(b_ps, lhsT=ones_row, rhs=srow,
                                 start=True, stop=True)
                bc = pmid.tile([128, TSH], BF, tag=tag, name=tag)
                nc.scalar.copy(bc, b_ps)
                return bc

            # kv latent: norm + roped kpe, packed into lat_stage
            zkv = psa.tile([1, TSH], F32, tag="z")
            kv_stages = []
            for m in range(KVMT):
                kv_stages.append(a_mtile(
                    lambda k, m=m: wkva_t[:, m, k * 128:(k + 1) * 128], 128,
                    zkv, m == 0, m == KVMT - 1, f"stkv{m}"))
            kpe_stage = a_mtile(lambda k: wkpe_t[:, k, :], DR,
                                None, False, False, "stkpe")
            skv_bc = rsqrt_bc(zkv, KVLR, "skvbc")
            for m in range(KVMT):
                nc.vector.tensor_tensor(
                    lat_stage[:, m * TSH:(m + 1) * TSH],
                    kv_stages[m], skv_bc, op=MUL)
            # kpe rope -> packed [128,128]: halves of the 256 tokens stacked
            sw_ps = psa.tile([DR, TSH], F32, tag="swp", bufs=2)
            nc.tensor.matmul(sw_ps, lhsT=perm_t[0:DR, 0:DR], rhs=kpe_stage,
                             start=True, stop=True)
            rt1 = pmid.tile([DR, TSH], BF, tag="rt1", bufs=2)
            nc.vector.tensor_tensor(rt1, kpe_stage, cosf_t[0:DR, :], op=MUL)
            rt2 = pmid.tile([DR, TSH], BF, tag="rt2", bufs=2)
            nc.vector.tensor_tensor(rt2, sw_ps, sinf_t[0:DR, :], op=MUL)
            half = TSH // 2
            nc.vector.tensor_tensor(
                lat_stage[0:DR, KVMT * TSH:KVMT * TSH + half],
                rt1[:, 0:half], rt2[:, 0:half], op=ADD)
            nc.vector.tensor_tensor(
                lat_stage[DR:128, KVMT * TSH:KVMT * TSH + half],
                rt1[:, half:TSH], rt2[:, half:TSH], op=ADD)
            nc.sync.dma_start(out=contrib_lat, in_=lat_stage)

            # q_a + norm
            zq = psa.tile([1, TSH], F32, tag="z")
            q_stages = []
            for m in range(QMT):
                q_stages.append(a_mtile(
                    lambda k, m=m: wqa_t[m // 4][:, m % 4, k * 128:(k + 1) * 128],
                    128, zq, m == 0, m == QMT - 1, f"stq{m}"))
            sq_bc = rsqrt_bc(zq, QLR, "sqbc")
            qan = []
            for m in range(QMT):
                qq = pmid.tile([128, TSH], BF, tag=f"qan{m}", name=f"qan{m}")
                nc.vector.tensor_tensor(qq, q_stages[m], sq_bc, op=MUL)
                qan.append(qq)

        # kv latent AllGather (outer scope! in-scope would gate SBUF reuse)
        nc.gpsimd.collective_compute(
            "AllGather", mybir.AluOpType.bypass,
            replica_groups=[list(range(NCORES))],
            ins=[contrib_lat], outs=[gath_lat])

        # ---- q_b projections for all dests ----------------------------------
        with tc.tile_pool(name="pw", bufs=1) as pw, \
             tc.tile_pool(name="psw", bufs=1, space="PSUM") as psw:
            for dg in range(4):
                wq = pw.tile([128, 2, QMT * 3 * 128], BF, tag="wq", bufs=2)
                nc.sync.dma_start(
                    out=wq,
                    in_=wqb[dg * 256:(dg + 1) * 256, :].rearrange(
                        "(n p) km -> p n km", n=2))
                for i in range(2):
                    d = 2 * dg + i
                    acc = []
                    for mt in range(3):
                        acc.append(psw.tile([128, TSH], F32, tag=f"acc{mt}",
                                            bufs=2, name=f"acc{mt}"))
                    for k in range(QMT):
                        for mt in range(3):
                            nc.tensor.matmul(
                                acc[mt],
                                lhsT=wq[:, i, k * 384 + mt * 128:
                                        k * 384 + (mt + 1) * 128],
                                rhs=qan[k],
                                start=(k == 0), stop=(k == QMT - 1))
                    for h in range(HPC):
                        nc.vector.tensor_copy(
                            q_stage[:, d * QFC + h * TSH:
                                    d * QFC + (h + 1) * TSH], acc[h])
                    qraw = pw.tile([128, TSH], BF, tag="qraw", bufs=2)
                    nc.vector.tensor_copy(qraw, acc[2])
                    sw_ps = psw.tile([128, TSH], F32, tag="swp", bufs=2)
                    nc.tensor.matmul(sw_ps, lhsT=perm_t, rhs=qraw,
                                     start=True, stop=True)
                    rt1 = pw.tile([128, TSH], BF, tag="rt1", bufs=2)
                    nc.vector.tensor_tensor(rt1, qraw, cosf_t, op=MUL)
                    rt2 = pw.tile([128, TSH], BF, tag="rt2", bufs=2)
                    nc.vector.tensor_tensor(rt2, sw_ps, sinf_t, op=MUL)
                    nc.vector.tensor_tensor(
                        q_stage[:, d * QFC + 2 * TSH:d * QFC + 3 * TSH],
                        rt1, rt2, op=ADD)
                # ship two dests per DMA as they complete
                nc.sync.dma_start(
                    out=contrib_q[dg * 256:(dg + 1) * 256, :].rearrange(
                        "(n p) f -> p n f", n=2),
                    in_=q_stage[:, dg * 2 * QFC:(dg + 1) * 2 * QFC].rearrange(
                        "p (n f) -> p n f", n=2))

        # q AllToAll (outer scope)
        nc.gpsimd.collective_compute(
            "AllToAll", mybir.AluOpType.bypass,
            replica_groups=[list(range(NCORES))],
            ins=[contrib_q], outs=[a2a_q])

        # ---- kv_b expansion for local heads over all tokens (overlaps a2a) --
        lat = []
        kn = []
        vt = []
        gath_sv = gath_lat.rearrange("(s p) f -> p s f", s=NCORES)
        with tc.tile_pool(name="pkb", bufs=1) as pkb, \
             tc.tile_pool(name="pskb", bufs=1, space="PSUM") as pskb:
            wkvb_t = pkb.tile([128, KVMT, 4 * DN], BF, tag="wkvb")
            nc.scalar.dma_start(
                out=wkvb_t,
                in_=wkvb.rearrange("p (kc m) -> p kc m", kc=KVMT))
            for k in range(KVMT):
                lt = bcp.tile([128, NCORES, TSH], BF, tag=f"lat{k}",
                              name=f"lat{k}")
                nc.scalar.dma_start(
                    out=lt, in_=gath_sv[:, :, k * TSH:(k + 1) * TSH])
                lat.append(lt)
            kpe_all = bcp.tile([DR, NCORES, 2, TSH // 2], BF, tag="kpeall",
                               name="kpeall")
            nc.scalar.dma_start(
                out=kpe_all,
                in_=gath_lat.rearrange("(s two pa) f -> pa s two f",
                                       s=NCORES, two=2)[
                    :, :, :, KVMT * TSH:KVMT * TSH + TSH // 2])
            kpe_flat = kpe_all.rearrange("p s two f -> p (s two f)")
            lat_flat = [lt.rearrange("p s f -> p (s f)") for lt in lat]

            for h in range(HPC):
                knt = bcp.tile([128, T], BF, tag=f"kn{h}", name=f"kn{h}")
                for c in range(NB):
                    ps = pskb.tile([128, 512], F32, tag="knps", bufs=2)
                    for k in range(KVMT):
                        nc.tensor.matmul(
                            ps, lhsT=wkvb_t[:, k, h * DN:(h + 1) * DN],
                            rhs=lat_flat[k][:, c * 512:(c + 1) * 512],
                            start=(k == 0), stop=(k == KVMT - 1))
                    if c % 2 == 0:
                        nc.vector.tensor_copy(knt[:, c * 512:(c + 1) * 512], ps)
                    else:
                        nc.scalar.copy(knt[:, c * 512:(c + 1) * 512], ps)
                kn.append(knt)
            for tb in range(TBT):
                v = bcp.tile([128, HPC * DV], BF, tag=f"v{tb}", name=f"v{tb}")
                ps = pskb.tile([128, HPC * DV], F32, tag="vps", bufs=3)
                for k in range(KVMT):
                    nc.tensor.matmul(
                        ps, lhsT=lat_flat[k][:, tb * 128:(tb + 1) * 128],
                        rhs=wkvb_t[:, k, 2 * DN:4 * DN],
                        start=(k == 0), stop=(k == KVMT - 1))
                if tb % 2 == 0:
                    nc.vector.tensor_copy(v, ps)
                else:
                    nc.scalar.copy(v, ps)
                vt.append(v)

        # ---- Phase B: head-sharded attention + output projection ------------
        a2a_sv = a2a_q.rearrange("(s p) f -> p s f", s=NCORES)
        a2a_pe = a2a_q.rearrange("(s two pa) f -> pa s two f", s=NCORES, two=2)
        qn2 = [None] * NB
        qpe2 = [None] * NB
        for qj in range(NB):
            qn2[qj] = bcp.tile([128, HPC, 2, TSH], BF, tag=f"qn{qj}",
                               name=f"qn{qj}")
            qpe2[qj] = bcp.tile([DR, HPC, 2, TSH], BF, tag=f"qpe{qj}",
                                name=f"qpe{qj}")
            for h in range(HPC):
                nc.sync.dma_start(
                    out=qn2[qj][:, h, :, :],
                    in_=a2a_sv[:, 2 * qj:2 * qj + 2,
                               h * TSH:(h + 1) * TSH])
                nc.sync.dma_start(
                    out=qpe2[qj][:, h, :, :],
                    in_=a2a_pe[:, 2 * qj:2 * qj + 2, h, 2 * TSH:3 * TSH])
        qn = [[qn2[qj].rearrange("p h s f -> p (h s f)")[:, h * 512:(h + 1) * 512]
               for qj in range(NB)] for h in range(HPC)]
        qpe = [[qpe2[qj].rearrange("p h s f -> p (h s f)")[:, h * 512:(h + 1) * 512]
                for qj in range(NB)] for h in range(HPC)]

        with tc.tile_pool(name="pc", bufs=1) as pc, \
             tc.tile_pool(name="psc", bufs=1, space="PSUM") as psc:
            attn_n = [None] * HPC
            for qj in range(NB):
                nki = 4 * qj + 4
                for h in range(HPC):
                    attn_ps = psc.tile([128, 512], F32, tag="attn", bufs=2)
                    z_ps = psc.tile([1, 512], F32, tag="zr", bufs=1)
                    for ki in range(nki):
                        ksl = slice(ki * 128, (ki + 1) * 128)
                        s_ps = psc.tile([128, 512], F32, tag="s", bufs=3)
                        nc.tensor.matmul(s_ps, lhsT=kn[h][:, ksl],
                                         rhs=qn[h][qj],
                                         start=True, stop=False)
                        nc.tensor.matmul(s_ps, lhsT=kpe_flat[:, ksl],
                                         rhs=qpe[h][qj],
                                         start=False, stop=True)
                        e = pc.tile([128, 512], BF, tag="e", bufs=4)
                        nc.scalar.activation(e, s_ps,
                                             mybir.ActivationFunctionType.Exp)
                        if ki >= 4 * qj:
                            sub_d = ki - 4 * qj
                            nc.vector.tensor_tensor(
                                e, e, maskd_t[:, sub_d * 512:(sub_d + 1) * 512],
                                op=MUL)
                        nc.tensor.matmul(z_ps, lhsT=ones_col, rhs=e,
                                         start=(ki == 0), stop=(ki == nki - 1))
                        nc.tensor.matmul(attn_ps,
                                         lhsT=vt[ki][:, h * DV:(h + 1) * DV],
                                         rhs=e,
                                         start=(ki == 0), stop=(ki == nki - 1))
                    rz = pc.tile([1, 512], BF, tag="rz", bufs=2)
                    with nc.allow_low_precision(reason="bf16 softmax denom"):
                        nc.vector.reciprocal(rz, z_ps)
                    bc_ps = psc.tile([128, 512], F32, tag="s", bufs=3)
                    nc.tensor.matmul(bc_ps, lhsT=ones_row, rhs=rz,
                                     start=True, stop=True)
                    bc_sb = pc.tile([128, 512], BF, tag="bcs", bufs=2)
                    nc.scalar.copy(bc_sb, bc_ps)
                    attn_n[h] = pc.tile([128, 512], BF, tag=f"attnn{h}",
                                        bufs=2, name=f"attnn{h}_{qj}")
                    nc.vector.tensor_tensor(attn_n[h], attn_ps, bc_sb, op=MUL)

                for tt in range(4):
                    tb = qj * 4 + tt
                    tsl = slice(tt * 128, (tt + 1) * 128)
                    o_row = pc.tile([128, HID], BF, tag="orow", bufs=2)
                    for hb in range(NB):
                        o_ps = psc.tile([128, 512], F32, tag="o", bufs=2)
                        for h in range(HPC):
                            nc.tensor.matmul(
                                o_ps,
                                lhsT=attn_n[h][:, tsl],
                                rhs=wo_t[h][:, hb * 512:(hb + 1) * 512],
                                start=(h == 0),
                                stop=(h == HPC - 1),
                            )
                        if hb % 2 == 0:
                            nc.vector.tensor_copy(
                                o_row[:, hb * 512:(hb + 1) * 512], o_ps)
                        else:
                            nc.scalar.copy(
                                o_row[:, hb * 512:(hb + 1) * 512], o_ps)
                    if tt % 2 == 0:
                        nc.scalar.dma_start(
                            out=out[tb * 128:(tb + 1) * 128, :], in_=o_row)
                    else:
                        nc.sync.dma_start(
                            out=out[tb * 128:(tb + 1) * 128, :], in_=o_row)


_NC_CACHE = {}


def _get_nc():
    if "nc" not in _NC_CACHE:
        _NC_CACHE["nc"] = build_bass()
    return _NC_CACHE["nc"]


def make_in_maps(positions, hidden_states, w_q_a, q_a_ln_w, w_q_b, w_kv_a,
                 kv_a_ln_w, w_kv_b, w_o):
    BF_NP = mybir.dt.np(mybir.dt.bfloat16)

    positions = np.asarray(positions)
    hidden_states = np.asarray(hidden_states, dtype=np.float32)
    w_q_a = np.asarray(w_q_a, dtype=np.float32)
    q_a_ln_w = np.asarray(q_a_ln_w, dtype=np.float32)
    w_q_b = np.asarray(w_q_b, dtype=np.float32)
    w_kv_a = np.asarray(w_kv_a, dtype=np.float32)
    kv_a_ln_w = np.asarray(kv_a_ln_w, dtype=np.float32)
    w_kv_b = np.asarray(w_kv_b, dtype=np.float32)
    w_o = np.asarray(w_o, dtype=np.float32)

    hs_t = np.ascontiguousarray(hidden_states.T)

    # deinterleave rope features: evens then odds (dot-products invariant)
    order = np.concatenate([np.arange(0, DR, 2), np.arange(1, DR, 2)])

    wkva_p = w_kv_a.copy()
    wkva_p[:, KVLR:] = w_kv_a[:, KVLR:][:, order]

    inv_freq = 1.0 / (THETA ** (np.arange(0, DR, 2, dtype=np.float64) / DR))
    ang = positions.astype(np.float64)[:, None] * inv_freq[None, :]
    cosT = np.cos(ang).T
    sinT = np.sin(ang).T
    cosf = np.concatenate([cosT, cosT], axis=0)        # [64, T]
    sinf = np.concatenate([-sinT, sinT], axis=0)       # [64, T]
    cosf2 = np.concatenate([cosf, cosf], axis=0)       # [128, T] dual-head
    sinf2 = np.concatenate([sinf, sinf], axis=0)

    perm64 = np.zeros((DR, DR), dtype=np.float32)
    for i in range(DR):
        perm64[i, (i + DR // 2) % DR] = 1.0
    perm128 = np.zeros((128, 128), dtype=np.float32)
    perm128[:DR, :DR] = perm64
    perm128[DR:, DR:] = perm64

    maskd = np.zeros((128, 4 * 512), dtype=np.float32)
    p = np.arange(128)[:, None]
    f = np.arange(512)[None, :]
    for sub in range(4):
        maskd[:, sub * 512:(sub + 1) * 512] = (p + 128 * sub <= f)

    # all-heads q_b weights, columns grouped per destination core:
    # [qn h0 (128) | qn h1 (128) | qpe h0 perm (64) | qpe h1 perm (64)]
    wqb_all = np.concatenate([
        np.concatenate([
            w_q_b[:, h0 * DQK:h0 * DQK + DN],
            w_q_b[:, h1 * DQK:h1 * DQK + DN],
            w_q_b[:, h0 * DQK + DN:(h0 + 1) * DQK][:, order],
            w_q_b[:, h1 * DQK + DN:(h1 + 1) * DQK][:, order],
        ], axis=1)
        for h0, h1 in ((2 * d, 2 * d + 1) for d in range(NCORES))
    ], axis=1) * q_a_ln_w[:, None] * SCALE

    def pack(w, mrows):
        # [K, M] -> strip-major [nstrips*128, (K/128)*mrows]: each strip row-
        # contiguous so the device DMA is 128 fat descriptors
        Kd, Md = w.shape
        n = Md // mrows
        return np.ascontiguousarray(
            w.reshape(Kd // 128, 128, n, mrows).transpose(2, 1, 0, 3)
            .reshape(n * 128, (Kd // 128) * mrows)).astype(BF_NP)

    wqa_pk = pack(w_q_a, 128)
    wkva_pk = pack(wkva_p[:, :KVLR], 128)
    wkpe_pk = pack(wkva_p[:, KVLR:], DR)
    wqb_pk = pack(wqb_all, HPC * DQK)

    in_maps = []
    for c in range(NCORES):
        h0, h1 = HPC * c, HPC * c + 1
        # per-core kv_b: cols [kn h0 | kn h1 | v h0 | v h1], ln folded
        wkvb_c = np.concatenate([
            w_kv_b[:, h0 * (DN + DV):h0 * (DN + DV) + DN],
            w_kv_b[:, h1 * (DN + DV):h1 * (DN + DV) + DN],
            w_kv_b[:, h0 * (DN + DV) + DN:(h0 + 1) * (DN + DV)],
            w_kv_b[:, h1 * (DN + DV) + DN:(h1 + 1) * (DN + DV)],
        ], axis=1) * kv_a_ln_w[:, None]
        wkvb_pk = pack(wkvb_c, 4 * DN)
        wo_c = np.concatenate([
            w_o[h0 * DV:(h0 + 1) * DV, :],
            w_o[h1 * DV:(h1 + 1) * DV, :],
        ], axis=0)
        tsl = slice(c * TSH, (c + 1) * TSH)
        in_maps.append({
            "hs_sh": np.ascontiguousarray(hs_t[:, tsl]).astype(BF_NP),
            "wqa": wqa_pk,
            "wkva": wkva_pk,
            "wkpe": wkpe_pk,
            "wqb": wqb_pk,
            "wkvb": wkvb_pk,
            "wo": np.ascontiguousarray(wo_c).astype(BF_NP),
            "cosf_sh": np.ascontiguousarray(cosf2[:, tsl]).astype(BF_NP),
            "sinf_sh": np.ascontiguousarray(sinf2[:, tsl]).astype(BF_NP),
            "perm128": perm128.astype(BF_NP),
            "maskd": np.ascontiguousarray(maskd).astype(BF_NP),
            "ones": np.ones((128, 128), dtype=np.float32).astype(BF_NP),
        })
    return in_maps


def kernel(positions, hidden_states, w_q_a, q_a_ln_w, w_q_b, w_kv_a,
           kv_a_ln_w, w_kv_b, w_o):
    nc = _get_nc()
    in_maps = make_in_maps(positions, hidden_states, w_q_a, q_a_ln_w, w_q_b,
                           w_kv_a, kv_a_ln_w, w_kv_b, w_o)
    res = bass_utils.run_bass_kernel_spmd(nc, in_maps, core_ids=list(range(NCORES)))
    acc = np.zeros((T, HID), dtype=np.float32)
    for c in range(NCORES):
        acc += np.asarray(res.results[c]["out"], dtype=np.float32)
    return acc
